# revision 1
# baseline (speedup 1.0000x reference)
"""BERT-base forward (B=16, S=512, D=768, H=12, L=12) on 8 Trainium2 NeuronCores.

Sharding: data-parallel over batch — each core runs 2 sequences (1024 tokens)
with a full replica of the weights. No collectives.

Device layout: "transposed activations" — activations live as x^T [D, tokens]
(features on SBUF partitions, tokens on the free dim), so every linear layer is
out^T = W^T.T @ x^T with the (host-pre-tiled) weight as the stationary operand
and 512-token chunks as the moving operand.

Precision: bf16 matmul operands (fast weight load, half DMA), fp32 PSUM
accumulation, fp32 residual stream xT with a bf16 shadow xTb feeding GEMMs.
LayerNorm statistics matmuls run in fp32r on the fp32 residual.

Attention per (seq, head): logits^T [s,t] via K/Q head slices (contraction=64),
exp on ACT straight out of 2-bank PSUM (no max subtraction: |logits| is O(1)
after LayerNorm), denominator via an all-ones stationary matmul (replicated
across partitions), attn@V with the full 128-row V^T-transposed tile so each
head's output lands on its own partitions, then one DVE multiply by the
reciprocal denominator.
"""
import sys
sys.path.insert(0, '/opt/trn_rl_repo')

import numpy as np
import ml_dtypes
import concourse.bass as bass
import concourse.tile as tile
from concourse import bacc, mybir
from concourse.bass_utils import run_bass_kernel_spmd

# Model shapes (hardcoded)
V = 30522
S = 512
D = 768
H = 12
L = 12
F = 3072
B = 16
HD = 64
EPS = 1e-12
SCALE = HD ** (-0.5)

NCORES = 8
B_LOC = B // NCORES          # 2 sequences per core
T = B_LOC * S                # 1024 tokens per core
KD = D // 128                # 6 k-tiles over D
QKVM = 3 * D // 128          # 18 m-tiles for qkv
FM = F // 128                # 24 m-tiles over mlp hidden
PAIRS = H // 2               # 6 head-pairs
TCH = 512                    # token chunk for all GEMMs
NT = T // TCH                # 2

F32 = mybir.dt.float32
F32R = mybir.dt.float32r
BF16 = mybir.dt.bfloat16
I32 = mybir.dt.int32
AF = mybir.ActivationFunctionType
OP = mybir.AluOpType

_CACHED_NC = None


def _host_tile_weight(w_t):
    """w_t: [dout, din] torch-Linear weight. Returns [m_tiles, 128, din] where
    slice [m] is (w_t.T)[:, m*128:(m+1)*128] laid out partition-major."""
    dout, din = w_t.shape
    m_tiles, k_tiles = dout // 128, din // 128
    a = np.ascontiguousarray(w_t.T)                      # [din, dout]
    a = a.reshape(k_tiles, 128, m_tiles, 128)            # [k, p, m, w]
    a = np.ascontiguousarray(a.transpose(2, 1, 0, 3))    # [m, p, k, w]
    return a.reshape(m_tiles, 128, din).astype(ml_dtypes.bfloat16)


def build_nc(n_layers=L):
    nc = bacc.Bacc("TRN2", target_bir_lowering=False, debug=False)

    def din(name, shape, dt=BF16):
        return nc.dram_tensor(name, shape, dt, kind="ExternalInput").ap()

    nl = max(1, n_layers)
    tokens = din("tokens", [T], I32)
    tok_emb = din("tok_emb", [V, D], F32R)
    possent = din("possent", [S, D], F32)
    embw = din("embw", [D], F32)
    embb = din("embb", [D], F32)
    wqkv = din("wqkv", [nl, QKVM, 128, D])
    bqkv = din("bqkv", [nl, 3 * D], F32)
    wproj = din("wproj", [nl, KD, 128, D])
    bproj = din("bproj", [nl, D], F32)
    w1 = din("w1", [nl, FM, 128, D])
    b1 = din("b1", [nl, F], F32)
    w2 = din("w2", [nl, KD, 128, F])
    b2 = din("b2", [nl, D], F32)
    ln1w = din("ln1w", [nl, D], F32)
    ln1b = din("ln1b", [nl, D], F32)
    ln2w = din("ln2w", [nl, D], F32)
    ln2b = din("ln2b", [nl, D], F32)
    wpool = din("wpool", [KD, 128, D], BF16)
    bpool = din("bpool", [D], F32)
    ident = din("ident", [128, 128], F32R)
    ones = din("ones", [128, 128], F32R)
    ones_bf = din("ones_bf", [128, 128], BF16)
    out = nc.dram_tensor("out", [B_LOC, D], F32, kind="ExternalOutput").ap()

    with tile.TileContext(nc) as tc:
        _build_body(nc, tc, n_layers, tokens, tok_emb, possent, embw, embb,
                    wqkv, bqkv, wproj, bproj, w1, b1, w2, b2,
                    ln1w, ln1b, ln2w, ln2b, wpool, bpool, ident, ones, ones_bf,
                    out)
    nc.compile()
    return nc


def _build_body(nc, tc, n_layers, tokens, tok_emb, possent, embw, embb,
                wqkv, bqkv, wproj, bproj, w1, b1, w2, b2,
                ln1w, ln1b, ln2w, ln2b, wpool, bpool, ident, ones, ones_bf,
                out):
    from contextlib import ExitStack
    ctx = ExitStack()
    with ctx:
        consts = ctx.enter_context(tc.tile_pool(name="consts", bufs=1))
        xpool = ctx.enter_context(tc.tile_pool(name="xpool", bufs=1))
        qkvpool = ctx.enter_context(tc.tile_pool(name="qkvpool", bufs=9))
        wpool6 = ctx.enter_context(tc.tile_pool(name="wpool6", bufs=8))
        psum = ctx.enter_context(tc.tile_pool(name="psum", bufs=8, space="PSUM"))

        ident_sb = consts.tile([128, 128], F32R)
        nc.sync.dma_start(ident_sb[:], ident)
        ones_sb = consts.tile([128, 128], F32R)
        nc.sync.dma_start(ones_sb[:], ones)
        onesb_sb = consts.tile([128, 128], BF16)
        nc.sync.dma_start(onesb_sb[:], ones_bf)
        eps_sb = consts.tile([128, 1], F32)
        nc.vector.memset(eps_sb[:], EPS)

        # residual stream x^T (fp32) + bf16 shadow for GEMM inputs
        xT = xpool.tile([128, KD, T], F32R)
        xTb = xpool.tile([128, KD, T], BF16)
        # attention output a^T (bf16: proj GEMM input)
        aT = xpool.tile([128, KD, T], BF16)

        def ps_mm():
            return psum.tile([128, TCH], F32, tag="mm", name="ps")

        # ---------------- Embedding ----------------
        with tc.tile_pool(name="embp", bufs=2) as embp:
            embw_sb = embp.tile([128, D], F32, tag="embw")
            nc.sync.dma_start(embw_sb[:], embw[None, :].to_broadcast([128, D]))
            for tt in range(T // 128):
                idx_sb = embp.tile([128, 1], I32, tag="idx")
                nc.sync.dma_start(idx_sb[:], tokens[tt * 128:(tt + 1) * 128, None])
                g_sb = embp.tile([128, D], F32R, tag="g")
                nc.gpsimd.indirect_dma_start(
                    out=g_sb[:], out_offset=None, in_=tok_emb,
                    in_offset=bass.IndirectOffsetOnAxis(ap=idx_sb[:, :1], axis=0))
                p_sb = embp.tile([128, D], F32, tag="p")
                prow = (tt * 128) % S
                nc.sync.dma_start(p_sb[:], possent[prow:prow + 128, :])
                nc.vector.tensor_add(g_sb[:], g_sb[:], p_sb[:])
                # LayerNorm over free dim (d): bn_stats in 2 subgroups of 384
                st_sb = embp.tile([128, 2, 6], F32, tag="st")
                gv = g_sb[:].rearrange("p (a b) -> p a b", a=2)
                for a in range(2):
                    nc.vector.bn_stats(st_sb[:, a, :], gv[:, a, :])
                mv = embp.tile([128, 2], F32, tag="mv")
                nc.vector.bn_aggr(mv[:], st_sb[:])
                sd = embp.tile([128, 1], F32, tag="sd")
                nc.scalar.activation(sd[:], mv[:, 1:2], AF.Sqrt, bias=eps_sb[:])
                nc.vector.reciprocal_approx_fast(sd[:], sd[:])
                nc.vector.tensor_scalar(g_sb[:], g_sb[:], mv[:, 0:1], sd[:],
                                        op0=OP.subtract, op1=OP.mult)
                nc.vector.tensor_mul(g_sb[:], g_sb[:], embw_sb[:])
                # transpose into xT (pre-bias: ln bias is folded into the l=0
                # qkv bias on the host; xT gets it below for the residual)
                for k in range(KD):
                    pst = psum.tile([128, 128], F32R, tag="mm", name="pst")
                    nc.tensor.transpose(pst[:], g_sb[:, k * 128:(k + 1) * 128],
                                        ident_sb[:])
                    nc.vector.tensor_copy(xT[:, k, tt * 128:(tt + 1) * 128], pst[:])
            nc.vector.tensor_copy(xTb[:], xT[:])
            embbk_sb = embp.tile([128, KD], F32, tag="embbk")
            nc.sync.dma_start(embbk_sb[:], embb.rearrange("(k p) -> p k", p=128))
            embb_bc = embbk_sb[:, :, None].to_broadcast([128, KD, T])
            nc.gpsimd.tensor_add(xT[:], xT[:], embb_bc)

        # ---------------- Layer norm helper (transposed layout) ----------------
        def layer_norm_T(lw_sb, lb_sb, lnp):
            # stats via all-ones matmuls on the fp32 residual. The critical
            # path produces the bf16 shadow WITHOUT the ln bias (folded into
            # the downstream GEMM bias on the host); xT gets bias+center off
            # the critical path on gpsimd.
            for tch in range(NT):
                tsl = slice(tch * TCH, (tch + 1) * TCH)
                ps_s = ps_mm()
                ps_q = ps_mm()
                for k in range(KD):
                    nc.tensor.matmul(ps_s[:], lhsT=ones_sb[:], rhs=xT[:, k, tsl],
                                     start=(k == 0), stop=(k == KD - 1))
                for k in range(KD):
                    sq = lnp.tile([128, TCH], F32R, tag="sq")
                    nc.gpsimd.tensor_mul(sq[:], xT[:, k, tsl], xT[:, k, tsl])
                    nc.tensor.matmul(ps_q[:], lhsT=ones_sb[:], rhs=sq[:],
                                     start=(k == 0), stop=(k == KD - 1))
                mean = lnp.tile([128, TCH], F32, tag="mean")
                nc.vector.tensor_scalar_mul(mean[:], ps_s[:], 1.0 / D)
                xs = xT[:, :, tsl]
                mean_bc3 = mean[:, None, :].to_broadcast([128, 3, TCH])
                # center: split DVE / gpsimd to halve the chain
                nc.vector.tensor_sub(xs[:, 0:3, :], xs[:, 0:3, :], mean_bc3)
                nc.gpsimd.tensor_sub(xs[:, 3:6, :], xs[:, 3:6, :], mean_bc3)
                var = lnp.tile([128, TCH], F32, tag="var")
                nc.vector.tensor_mul(var[:], mean[:], mean[:])
                nc.vector.scalar_tensor_tensor(var[:], in0=ps_q[:], scalar=1.0 / D,
                                               in1=var[:], op0=OP.mult,
                                               op1=OP.subtract)
                nc.scalar.activation(var[:], var[:], AF.Sqrt, bias=eps_sb[:])
                r = lnp.tile([128, TCH], F32, tag="r")
                nc.vector.reciprocal_approx_fast(r[:], var[:])
                r_bc = r[:, None, :].to_broadcast([128, 1, TCH])
                # fused (xc*w)*r -> bf16 shadow, per k-tile, DVE/gpsimd split
                for k in range(KD):
                    nc.vector.scalar_tensor_tensor(
                        xTb[:, k, tsl], in0=xs[:, k, :],
                        scalar=lw_sb[:, k:k + 1], in1=r_bc[:, 0, :],
                        op0=OP.mult, op1=OP.mult)
                # xT = full ln output (shadow * 1 + b would lose fp32; redo in
                # fp32 on gpsimd off the critical path)
                b_bc = lb_sb[:, :, None].to_broadcast([128, KD, TCH])
                w_bc = lw_sb[:, :, None].to_broadcast([128, KD, TCH])
                r_bc6 = r[:, None, :].to_broadcast([128, KD, TCH])
                nc.gpsimd.tensor_mul(xs, xs, r_bc6)
                nc.gpsimd.tensor_mul(xs, xs, w_bc)
                nc.gpsimd.tensor_add(xs, xs, b_bc)

        # ---------------- Layers ----------------
        lctx = ExitStack()
        hpool = lctx.enter_context(tc.tile_pool(name="hpool", bufs=1))
        wpool24 = lctx.enter_context(tc.tile_pool(name="wpool24", bufs=2))
        biasp = lctx.enter_context(tc.tile_pool(name="biasp", bufs=2))
        attnp = lctx.enter_context(tc.tile_pool(name="attnp", bufs=4))
        lnp_pool = lctx.enter_context(tc.tile_pool(name="lnp", bufs=2))
        for l in range(n_layers):
            # per-layer bias/ln tiles
            bq_sb = biasp.tile([128, QKVM], F32, tag="bq")
            nc.sync.dma_start(bq_sb[:], bqkv[l].rearrange("(m p) -> p m", p=128))
            bp_sb = biasp.tile([128, KD], F32, tag="bp")
            nc.sync.dma_start(bp_sb[:], bproj[l].rearrange("(m p) -> p m", p=128))
            b1_sb = biasp.tile([128, FM], F32, tag="b1")
            nc.sync.dma_start(b1_sb[:], b1[l].rearrange("(m p) -> p m", p=128))
            b2_sb = biasp.tile([128, KD], F32, tag="b2")
            nc.sync.dma_start(b2_sb[:], b2[l].rearrange("(m p) -> p m", p=128))
            l1w_sb = biasp.tile([128, KD], F32, tag="l1w")
            nc.sync.dma_start(l1w_sb[:], ln1w[l].rearrange("(k p) -> p k", p=128))
            l1b_sb = biasp.tile([128, KD], F32, tag="l1b")
            nc.sync.dma_start(l1b_sb[:], ln1b[l].rearrange("(k p) -> p k", p=128))
            l2w_sb = biasp.tile([128, KD], F32, tag="l2w")
            nc.sync.dma_start(l2w_sb[:], ln2w[l].rearrange("(k p) -> p k", p=128))
            l2b_sb = biasp.tile([128, KD], F32, tag="l2b")
            nc.sync.dma_start(l2b_sb[:], ln2b[l].rearrange("(k p) -> p k", p=128))

            # ---- QKV + attention, interleaved by head-pair ----
            for pr in range(PAIRS):
                qkt = {}
                for mi, m in enumerate((pr, 6 + pr, 12 + pr)):
                    w_sb = wpool6.tile([128, D], BF16, tag="w6")
                    nc.sync.dma_start(w_sb[:], wqkv[l, m])
                    t_sb = qkvpool.tile([128, T], BF16, tag="qkv")
                    for tch in range(NT):
                        tsl = slice(tch * TCH, (tch + 1) * TCH)
                        ps = ps_mm()
                        for k in range(KD):
                            nc.tensor.matmul(ps[:], lhsT=w_sb[:, k * 128:(k + 1) * 128],
                                             rhs=xTb[:, k, tsl],
                                             start=(k == 0), stop=(k == KD - 1))
                        nc.scalar.activation(t_sb[:, tsl], ps[:], AF.Identity,
                                             bias=bq_sb[:, m:m + 1])
                    qkt[mi] = t_sb
                qt, kt, vt = qkt[0], qkt[1], qkt[2]
                # V transposes (DMA, bf16 128x128 tiles)
                vts_all = {}
                for s in range(B_LOC):
                    s0 = s * S
                    vts = attnp.tile([128, 4, 128], BF16, tag="vt", name="vts")
                    for st in range(4):
                        nc.scalar.dma_start(
                            vts[:, st, :],
                            vt[:, s0 + st * 128: s0 + (st + 1) * 128],
                            transpose=True)
                    vts_all[s] = vts
                # QK + exp for all (seq, head) first so PE never waits on ACT
                expP_all = {}
                for s in range(B_LOC):
                    s0 = s * S
                    for e in range(2):
                        po = 64 * e
                        expP = attnp.tile([128, 4, S], BF16, tag="expP",
                                          name="expP")
                        for st in range(4):
                            psl = ps_mm()
                            nc.tensor.matmul(
                                psl[:],
                                lhsT=kt[po:po + 64,
                                        s0 + st * 128: s0 + (st + 1) * 128],
                                rhs=qt[po:po + 64, s0:s0 + S],
                                start=True, stop=True)
                            nc.scalar.activation(expP[:, st, :], psl[:],
                                                 AF.Exp, scale=SCALE)
                        expP_all[(s, e)] = expP
                for s in range(B_LOC):
                    s0 = s * S
                    vts = vts_all[s]
                    u_pair = attnp.tile([128, S], F32, tag="u")
                    den_pair = attnp.tile([128, S], F32, tag="den")
                    rec_pair = attnp.tile([128, S], F32, tag="rec")
                    for e in range(2):
                        po = 64 * e
                        expP = expP_all[(s, e)]
                        psu = ps_mm()
                        for st in range(4):
                            nc.tensor.matmul(psu[:], lhsT=vts[:, st, :],
                                             rhs=expP[:, st, :],
                                             start=(st == 0), stop=(st == 3))
                        psd = ps_mm()
                        for st in range(4):
                            nc.tensor.matmul(psd[:], lhsT=onesb_sb[:],
                                             rhs=expP[:, st, :],
                                             start=(st == 0), stop=(st == 3))
                        nc.scalar.activation(u_pair[po:po + 64, :],
                                             psu[po:po + 64, :], AF.Identity)
                        nc.scalar.activation(den_pair[po:po + 64, :],
                                             psd[po:po + 64, :], AF.Identity)
                    nc.vector.reciprocal_approx_fast(rec_pair[:], den_pair[:])
                    nc.vector.tensor_mul(aT[:, pr, s0:s0 + S], u_pair[:],
                                         rec_pair[:])

            # ---- proj + residual into xT ----
            for m in range(KD):
                w_sb = wpool6.tile([128, D], BF16, tag="w6")
                nc.sync.dma_start(w_sb[:], wproj[l, m])
                for tch in range(NT):
                    tsl = slice(tch * TCH, (tch + 1) * TCH)
                    ps = ps_mm()
                    for k in range(KD):
                        nc.tensor.matmul(ps[:], lhsT=w_sb[:, k * 128:(k + 1) * 128],
                                         rhs=aT[:, k, tsl],
                                         start=(k == 0), stop=(k == KD - 1))
                    nc.vector.scalar_tensor_tensor(
                        xT[:, m, tsl], in0=ps[:], scalar=bp_sb[:, m:m + 1],
                        in1=xT[:, m, tsl], op0=OP.add, op1=OP.add)

            layer_norm_T(l1w_sb, l1b_sb, lnp_pool)

            # ---- MLP: h kept bf16 across full T so w1/w2 load once ----
            h = hpool.tile([128, FM, T], BF16, tag="h")
            for m in range(FM):
                w_sb = wpool6.tile([128, D], BF16, tag="w6")
                nc.sync.dma_start(w_sb[:], w1[l, m])
                for tch in range(NT):
                    tsl = slice(tch * TCH, (tch + 1) * TCH)
                    ps = ps_mm()
                    for k in range(KD):
                        nc.tensor.matmul(ps[:],
                                         lhsT=w_sb[:, k * 128:(k + 1) * 128],
                                         rhs=xTb[:, k, tsl],
                                         start=(k == 0), stop=(k == KD - 1))
                    nc.scalar.activation(h[:, m, tsl], ps[:], AF.Gelu,
                                         bias=b1_sb[:, m:m + 1])
            for m in range(KD):
                w2_sb = wpool24.tile([128, F], BF16, tag="w24")
                nc.sync.dma_start(w2_sb[:], w2[l, m])
                for tch in range(NT):
                    tsl = slice(tch * TCH, (tch + 1) * TCH)
                    ps = ps_mm()
                    for k in range(FM):
                        nc.tensor.matmul(ps[:],
                                         lhsT=w2_sb[:, k * 128:(k + 1) * 128],
                                         rhs=h[:, k, tsl],
                                         start=(k == 0), stop=(k == FM - 1))
                    nc.vector.scalar_tensor_tensor(
                        xT[:, m, tsl], in0=ps[:], scalar=b2_sb[:, m:m + 1],
                        in1=xT[:, m, tsl], op0=OP.add, op1=OP.add)

            layer_norm_T(l2w_sb, l2b_sb, lnp_pool)
        lctx.close()

        # ---------------- Pooler ----------------
        with tc.tile_pool(name="poolp", bufs=1) as poolp:
            bpl_sb = poolp.tile([128, KD], F32)
            nc.sync.dma_start(bpl_sb[:], bpool.rearrange("(m p) -> p m", p=128))
            pool_sb = poolp.tile([128, KD, B_LOC], F32R)
            for m in range(KD):
                w_sb = poolp.tile([128, D], BF16, tag="wp", name="w_sb")
                nc.sync.dma_start(w_sb[:], wpool[m])
                ps = ps_mm()
                for k in range(KD):
                    first_tok = xTb[:, k, :].rearrange("p (b s) -> p b s", s=S)
                    nc.tensor.matmul(ps[:, :B_LOC],
                                     lhsT=w_sb[:, k * 128:(k + 1) * 128],
                                     rhs=first_tok[:, :, 0:1],
                                     start=(k == 0), stop=(k == KD - 1))
                nc.scalar.activation(pool_sb[:, m, :], ps[:, :B_LOC], AF.Tanh,
                                     bias=bpl_sb[:, m:m + 1])
            out_sb = poolp.tile([128, D], F32)
            for k in range(KD):
                pst = psum.tile([128, 128], F32R, tag="mm", name="pst")
                nc.tensor.transpose(pst[:B_LOC, :], pool_sb[:, k, :], ident_sb[:])
                nc.vector.tensor_copy(out_sb[:B_LOC, k * 128:(k + 1) * 128],
                                      pst[:B_LOC, :])
            nc.sync.dma_start(out, out_sb[:B_LOC, :])


def _prep_host(inputs, n_layers=L):
    f32 = lambda a: np.asarray(a, dtype=np.float32)
    tokens = np.asarray(inputs["tokens"]).astype(np.int32)          # [16, 512]
    possent = f32(inputs["pos_emb"])[0] + f32(inputs["sent_emb"])[0, 0][None, :]

    def tile_stack(w, n):  # w: [L, dout, din]
        n = max(1, n)
        return np.stack([_host_tile_weight(f32(w[i])) for i in range(n)])

    nl = max(1, n_layers)
    # Fold layer-norm biases into the downstream GEMM bias (the device's bf16
    # shadow xTb deliberately omits the ln bias):
    #   qkv_b[l]  += qkv_w[l]  @ prev_ln_b   (emb_ln_b for l=0, ln2_b[l-1] else)
    #   mlp_b1[l] += mlp_w1[l] @ ln1_b[l]
    #   pool_b    += pool_w    @ ln2_b[last]
    qkv_b = f32(inputs["qkv_b"]).copy()
    mlp_b1 = f32(inputs["mlp_b1"]).copy()
    pool_b = f32(inputs["pool_b"]).copy()
    emb_ln_b = f32(inputs["emb_ln_b"])
    ln1_b = f32(inputs["ln1_b"])
    ln2_b = f32(inputs["ln2_b"])
    for l in range(nl):
        prev_b = emb_ln_b if l == 0 else ln2_b[l - 1]
        qkv_b[l] = qkv_b[l] + f32(inputs["qkv_w"][l]) @ prev_b
        mlp_b1[l] = mlp_b1[l] + f32(inputs["mlp_w1"][l]) @ ln1_b[l]
    if n_layers >= 1:
        pool_b = pool_b + f32(inputs["pool_w"]) @ ln2_b[nl - 1]
    else:
        pool_b = pool_b + f32(inputs["pool_w"]) @ emb_ln_b
    common = {
        "tok_emb": f32(inputs["tok_emb"]),
        "possent": possent.astype(np.float32),
        "embw": f32(inputs["emb_ln_w"]),
        "embb": f32(inputs["emb_ln_b"]),
        "wqkv": tile_stack(inputs["qkv_w"], n_layers),
        "bqkv": qkv_b[:nl],
        "wproj": tile_stack(inputs["proj_w"], n_layers),
        "bproj": f32(inputs["proj_b"])[:nl],
        "w1": tile_stack(inputs["mlp_w1"], n_layers),
        "b1": mlp_b1[:nl],
        "w2": tile_stack(inputs["mlp_w2"], n_layers),
        "b2": f32(inputs["mlp_b2"])[:nl],
        "ln1w": f32(inputs["ln1_w"])[:nl],
        "ln1b": f32(inputs["ln1_b"])[:nl],
        "ln2w": f32(inputs["ln2_w"])[:nl],
        "ln2b": f32(inputs["ln2_b"])[:nl],
        "wpool": _host_tile_weight(f32(inputs["pool_w"])),
        "bpool": pool_b,
        "ident": np.eye(128, dtype=np.float32),
        "ones": np.ones((128, 128), dtype=np.float32),
        "ones_bf": np.ones((128, 128), dtype=ml_dtypes.bfloat16),
    }
    in_maps = []
    for c in range(NCORES):
        m = dict(common)
        m["tokens"] = np.ascontiguousarray(
            tokens[c * B_LOC:(c + 1) * B_LOC].reshape(-1))
        in_maps.append(m)
    return in_maps


def kernel(**inputs) -> np.ndarray:
    global _CACHED_NC
    if _CACHED_NC is None:
        _CACHED_NC = build_nc(L)
    in_maps = _prep_host(inputs, L)
    res = run_bass_kernel_spmd(_CACHED_NC, in_maps,
                               core_ids=list(range(NCORES)), trace=False)
    return np.concatenate([res.results[c]["out"] for c in range(NCORES)], axis=0)

